# revision 1
# baseline (speedup 1.0000x reference)
"""RAFT-style CorrBlock kernel for Trainium2 (8 NeuronCores, Bass/Tile).

Full inputs: fmap1 [2,256,64,64], fmap2 [2,256,64,64], centroids_coords [2,2,64,64].
Output: [2, 324, 64, 64] f32.

Sharding: data-parallel over the B*H1*W1 query-pixel axis. Core c handles batch
c//4, query pixels (c%4)*1024 .. +1024. Each core:
  - corr rows via PE matmul f1_chunk^T @ f2 (and @ pooled f2 for pyramid levels
    1-3; avg-pooling commutes with the channel contraction),
  - writes the per-pixel 4-level pyramid slab to DRAM scratch,
  - one indirect-DMA "band" gather per (pixel, level): 9*W+10 contiguous floats
    cover the whole 10x10 integer-aligned patch at floor(cc/2^l) - 4,
  - masks out-of-bounds patch entries, then combines 4 shifted 9x9 views with
    per-pixel scalar weights (bilinear frac weights * 1/16 * 0.25^l folded in),
  - writes [1024, 324] feats; host assembles/transposes the full output.
"""

import numpy as np

import concourse.bass as bass
import concourse.bacc as bacc
import concourse.mybir as mybir
import concourse.tile as tile
from concourse.bass_utils import run_bass_kernel_spmd

f32 = mybir.dt.float32
i32 = mybir.dt.int32
OP = mybir.AluOpType

P = 128
C = 256
HW = 4096          # h2*w2 at level 0
NPIX = 1024        # query pixels per core
NG = NPIX // P     # 8 groups of 128 pixels
NLVL = 4
S = 9              # sample window side (2*RADIUS+1)
PS = 10            # patch side
W_L = [64, 32, 16, 8]
HW_L = [w * w for w in W_L]           # 4096, 1024, 256, 64
B_L = [9 * w + PS for w in W_L]       # band length: 586, 298, 154, 82
BASE_L = [0]
for _l in range(1, NLVL):
    BASE_L.append(BASE_L[-1] + NPIX * HW_L[_l - 1])
TOT = BASE_L[-1] + NPIX * HW_L[-1]    # 1024*5440
G = 1024                              # zeroed guard elements at both slab ends
NT = G + TOT + G
FEAT = NLVL * S * S                   # 324


def _ap_view(t_ap, offset, dims):
    """Arbitrary strided view of a tile AP: dims = [[step, count], ...] free dims."""
    return bass.AP(t_ap.tensor, t_ap.offset + offset, [list(t_ap.ap[0])] + dims)


def build_bass():
    nc = bacc.Bacc("TRN2", target_bir_lowering=False, debug=False)

    f1_d = nc.dram_tensor("f1", [C, NPIX], f32, kind="ExternalInput")
    f2_d = nc.dram_tensor("f2", [C, HW], f32, kind="ExternalInput")
    ccx_d = nc.dram_tensor("ccx", [P, NG], f32, kind="ExternalInput")
    ccy_d = nc.dram_tensor("ccy", [P, NG], f32, kind="ExternalInput")
    out_d = nc.dram_tensor("out", [NPIX, FEAT], f32, kind="ExternalOutput")
    slab_d = nc.dram_tensor("slab", [NT], f32)  # Internal scratch

    with tile.TileContext(nc) as tc:
        with (
            tc.tile_pool(name="persist", bufs=1) as pp,
            tc.tile_pool(name="grp", bufs=2) as pg,
            tc.tile_pool(name="psum", bufs=8, space="PSUM") as ps,
            tc.tile_pool(name="post", bufs=1) as po,
        ):
            # ---- guard zero-fill ----
            zt = pp.tile([1, G], f32, tag="zt")
            nc.vector.memset(zt[:], 0.0)
            nc.sync.dma_start(slab_d.ap()[0:G][None, :], zt[:])
            nc.sync.dma_start(slab_d.ap()[NT - G:NT][None, :], zt[:])

            # ---- input loads ----
            f1t = []
            f2t = []
            for k in range(2):
                t1 = pp.tile([P, NPIX], f32, tag=f"f1_{k}")
                nc.sync.dma_start(t1[:], f1_d.ap()[k * P:(k + 1) * P, :])
                f1t.append(t1)
                t2 = pp.tile([P, HW], f32, tag=f"f2_{k}")
                nc.sync.dma_start(t2[:], f2_d.ap()[k * P:(k + 1) * P, :])
                f2t.append(t2)
            ccx = pp.tile([P, NG], f32, tag="ccx")
            ccy = pp.tile([P, NG], f32, tag="ccy")
            nc.sync.dma_start(ccx[:], ccx_d.ap())
            nc.sync.dma_start(ccy[:], ccy_d.ap())

            # ---- pool f2 spatially (sums, not means; scale folded into weights) ----
            # f2l[l][k] : [128, HW_L[l]] viewed as [H_l, W_l] row-major
            f2l = [f2t]
            for l in range(1, NLVL):
                w_in = W_L[l - 1]
                w_out = W_L[l]
                cur = []
                for k in range(2):
                    src = f2l[l - 1][k]
                    tmpx = pg.tile([P, w_in * w_out], f32, tag=f"f2pool_tmp_{l}")
                    # pool x: tmpx[y, x'] = src[y, 2x'] + src[y, 2x'+1]
                    nc.vector.tensor_tensor(
                        out=tmpx[:],
                        in0=_ap_view(src[:], 0, [[w_in, w_in], [2, w_out]]),
                        in1=_ap_view(src[:], 1, [[w_in, w_in], [2, w_out]]),
                        op=OP.add,
                    )
                    dst = pp.tile([P, w_out * w_out], f32, tag=f"f2l_{l}_{k}")
                    # pool y: dst[y', x'] = tmpx[2y'] + tmpx[2y'+1]
                    nc.gpsimd.tensor_tensor(
                        out=dst[:],
                        in0=_ap_view(tmpx[:], 0, [[2 * w_out, w_out], [1, w_out]]),
                        in1=_ap_view(tmpx[:], w_out, [[2 * w_out, w_out], [1, w_out]]),
                        op=OP.add,
                    )
                    cur.append(dst)
                f2l.append(cur)

            # ---- per-level index / weight / mask precompute ----
            # iota over patch coordinate k (0..9), replicated per group
            kvi = pp.tile([P, NG * PS], i32, tag="kvi")
            nc.gpsimd.iota(kvi[:], pattern=[[0, NG], [1, PS]], base=0, channel_multiplier=0)
            kvf = pp.tile([P, NG * PS], f32, tag="kvf")
            nc.vector.tensor_copy(out=kvf[:], in_=kvi[:])
            # pixel index n = g*128 + p as f32 (iota steps must fit int16)
            npix_i = pp.tile([P, NG], i32, tag="npix_i")
            nc.gpsimd.iota(npix_i[:], pattern=[[P, NG]], base=0, channel_multiplier=1)
            npix_f = pp.tile([P, NG], f32, tag="npix_f")
            nc.vector.tensor_copy(out=npix_f[:], in_=npix_i[:])

            idx_l = []
            w_l = []     # [w00, w01, w10, w11] per level, each [P, NG]
            m_l = []     # [P, NG*100] patch validity masks
            for l in range(NLVL):
                wl = W_L[l]
                inv = 1.0 / (1 << l)
                sc = 1.0 / (16.0 * (4.0 ** l))

                xs = pg.tile([P, NG], f32, tag="xs")
                ys = pg.tile([P, NG], f32, tag="ys")
                nc.vector.tensor_scalar_mul(xs[:], ccx[:], inv)
                nc.vector.tensor_scalar_mul(ys[:], ccy[:], inv)

                def floor_of(v, nm):
                    ti = pg.tile([P, NG], i32, tag=f"fl_i_{nm}")
                    nc.vector.tensor_copy(out=ti[:], in_=v[:])
                    tf = pg.tile([P, NG], f32, tag=f"fl_f_{nm}")
                    nc.vector.tensor_copy(out=tf[:], in_=ti[:])
                    gt = pg.tile([P, NG], f32, tag=f"fl_g_{nm}")
                    nc.vector.tensor_tensor(out=gt[:], in0=tf[:], in1=v[:], op=OP.is_gt)
                    fl = pg.tile([P, NG], f32, tag=f"fl_o_{nm}")
                    nc.vector.tensor_tensor(out=fl[:], in0=tf[:], in1=gt[:], op=OP.subtract)
                    return fl

                x0 = floor_of(xs, "x")
                y0 = floor_of(ys, "y")

                fx = pg.tile([P, NG], f32, tag="fx")
                fy = pg.tile([P, NG], f32, tag="fy")
                nc.vector.tensor_tensor(out=fx[:], in0=xs[:], in1=x0[:], op=OP.subtract)
                nc.vector.tensor_tensor(out=fy[:], in0=ys[:], in1=y0[:], op=OP.subtract)

                # weights: w_ab = wy_a * wx_b * sc ; wx1 = fx, wx0 = 1-fx
                wy0s = pg.tile([P, NG], f32, tag="wy0s")
                wy1s = pg.tile([P, NG], f32, tag="wy1s")
                # wy0*sc = (fy*-sc)+sc ; wy1*sc = fy*sc
                nc.vector.tensor_scalar(wy0s[:], fy[:], -sc, sc, OP.mult, OP.add)
                nc.vector.tensor_scalar_mul(wy1s[:], fy[:], sc)
                wx0 = pg.tile([P, NG], f32, tag="wx0")
                nc.vector.tensor_scalar(wx0[:], fx[:], -1.0, 1.0, OP.mult, OP.add)
                ws = []
                for a, wya in ((0, wy0s), (1, wy1s)):
                    for b, wxb in ((0, wx0), (1, fx)):
                        wt = pp.tile([P, NG], f32, tag=f"w{a}{b}_{l}")
                        nc.vector.tensor_tensor(out=wt[:], in0=wya[:], in1=wxb[:], op=OP.mult)
                        ws.append(wt)
                w_l.append(ws)

                # band start index: n*HW_l + (y0-4)*W_l + (x0-4) + BASE_l
                t1 = pg.tile([P, NG], f32, tag="idx_t1")
                nc.vector.scalar_tensor_tensor(
                    out=t1[:], in0=y0[:], scalar=float(wl), in1=x0[:],
                    op0=OP.mult, op1=OP.add,
                )
                t2 = pg.tile([P, NG], f32, tag="idx_t2")
                nc.vector.scalar_tensor_tensor(
                    out=t2[:], in0=npix_f[:], scalar=float(HW_L[l]), in1=t1[:],
                    op0=OP.mult, op1=OP.add,
                )
                t3 = pg.tile([P, NG], f32, tag="idx_t3")
                nc.vector.tensor_scalar_add(t3[:], t2[:], float(BASE_L[l] - 4 * wl - 4))
                ii = pp.tile([P, NG], i32, tag=f"idx_{l}")
                nc.vector.tensor_copy(out=ii[:], in_=t3[:])
                idx_l.append(ii)

                # row/col validity: valid iff 4-k <= c0 <= H+3-k  (c0=y0 or x0)
                def valid(c0, lim, nm):
                    # t = c0 + k   (broadcast c0 over k)
                    tt = pg.tile([P, NG * PS], f32, tag=f"v_t_{nm}")
                    nc.vector.tensor_tensor(
                        out=tt[:].rearrange("p (g k) -> p g k", k=PS),
                        in0=kvf[:].rearrange("p (g k) -> p g k", k=PS),
                        in1=c0[:, :, None].to_broadcast([P, NG, PS]),
                        op=OP.add,
                    )
                    c1 = pg.tile([P, NG * PS], f32, tag=f"v_c_{nm}")
                    nc.vector.tensor_scalar(c1[:], tt[:], 4.0, None, OP.is_ge)
                    vv = pg.tile([P, NG * PS], f32, tag=f"v_o_{nm}")
                    nc.vector.scalar_tensor_tensor(
                        out=vv[:], in0=tt[:], scalar=float(lim + 3), in1=c1[:],
                        op0=OP.is_le, op1=OP.mult,
                    )
                    return vv

                rv = valid(y0, wl, "r")
                cv = valid(x0, wl, "c")
                mm = pp.tile([P, NG * PS * PS], f32, tag=f"m_{l}")
                nc.vector.tensor_tensor(
                    out=mm[:].rearrange("p (g a b) -> p g a b", a=PS, b=PS),
                    in0=rv[:].rearrange("p (g k) -> p g k", k=PS)[:, :, :, None]
                        .to_broadcast([P, NG, PS, PS]),
                    in1=cv[:].rearrange("p (g k) -> p g k", k=PS)[:, :, None, :]
                        .to_broadcast([P, NG, PS, PS]),
                    op=OP.mult,
                )
                m_l.append(mm)

            # ---- per-group: matmuls -> PSUM -> SBUF -> slab writes ----
            import os
            _skip_mm = os.environ.get("K_SKIP_MM") == "1"
            _skip_post = os.environ.get("K_SKIP_POST") == "1"
            ncopy = 0
            for g in range(NG) if not _skip_mm else []:
                corr = [
                    pg.tile([P, HW_L[l]], f32, tag=f"corr_{l}", name=f"corr_{l}_{g}")
                    for l in range(NLVL)
                ]
                for l in range(NLVL):
                    rhs_src = f2l[l]
                    hwl = HW_L[l]
                    nfree = min(512, hwl)
                    for n in range(hwl // nfree):
                        pt = ps.tile([P, 512], f32, tag="mm")
                        for k in range(2):
                            nc.tensor.matmul(
                                out=pt[:, :nfree],
                                lhsT=f1t[k][:, g * P:(g + 1) * P],
                                rhs=rhs_src[k][:, n * nfree:(n + 1) * nfree],
                                start=(k == 0),
                                stop=(k == 1),
                            )
                        dst = corr[l][:, n * nfree:(n + 1) * nfree]
                        if ncopy % 2 == 0:
                            nc.scalar.copy(out=dst, in_=pt[:, :nfree])
                        else:
                            nc.vector.tensor_copy(out=dst, in_=pt[:, :nfree])
                        ncopy += 1
                for l in range(NLVL):
                    ofs = G + BASE_L[l] + g * P * HW_L[l]
                    nc.sync.dma_start(
                        slab_d.ap()[ofs:ofs + P * HW_L[l]].rearrange("(p f) -> p f", f=HW_L[l]),
                        corr[l][:],
                    )

            # ---- band gathers + mask + bilinear combine ----
            feats = po.tile([P, NG * FEAT], f32, tag="feats")
            if _skip_post:
                nc.vector.memset(feats[:], 0.0)
            for l in range(NLVL) if not _skip_post else []:
                bl = B_L[l]
                wl = W_L[l]
                band = po.tile([P, NG * bl], f32, tag=f"band_{l}")
                # HW DGE only honors one offset per partition -> one gather per group
                if os.environ.get("K_SKIP_GATHER") == "1":
                    nc.vector.memset(band[:], 0.0)
                else:
                    for g in range(NG):
                        nc.gpsimd.indirect_dma_start(
                            out=band[:, g * bl:(g + 1) * bl],
                            out_offset=None,
                            in_=slab_d.ap()[:, None],
                            in_offset=bass.IndirectOffsetOnAxis(ap=idx_l[l][:, g:g + 1], axis=0),
                            element_offset=G,
                        )
                if os.environ.get("K_SKIP_CMB") == "1":
                    continue
                # masked 10x10 patches, contiguous [P, NG*100]
                pm = po.tile([P, NG * PS * PS], f32, tag=f"pm_{l}")
                nc.vector.tensor_tensor(
                    out=pm[:].rearrange("p (g a b) -> p g a b", a=PS, b=PS),
                    in0=_ap_view(band[:], 0, [[bl, NG], [wl, PS], [1, PS]]),
                    in1=m_l[l][:].rearrange("p (g a b) -> p g a b", a=PS, b=PS),
                    op=OP.mult,
                )
                # reference: sample (i, j) is at x = cc_x + (i-4), y = cc_y + (j-4),
                # so out_tap[i, j] = sum_ab w_ab * patch[y=j+a, x=i+b]
                ov = _ap_view(feats[:], l * S * S, [[FEAT, NG], [S, S], [1, S]])
                for t, (a, b) in enumerate(((0, 0), (0, 1), (1, 0), (1, 1))):
                    pv = _ap_view(pm[:], a * PS + b, [[PS * PS, NG], [1, S], [PS, S]])
                    wb = w_l[l][t][:, :, None, None].to_broadcast([P, NG, S, S])
                    _cmb = nc.gpsimd if os.environ.get("K_CMB_ENG") == "gpsimd" else nc.vector
                    if t == 0:
                        nc.vector.tensor_tensor(out=ov, in0=pv, in1=wb, op=OP.mult)
                    else:
                        tmp = po.tile([P, NG * S * S], f32, tag=f"cmb_tmp")
                        tv = tmp[:].rearrange("p (g a b) -> p g a b", a=S, b=S)
                        _cmb.tensor_tensor(out=tv, in0=pv, in1=wb, op=OP.mult)
                        nc.vector.tensor_tensor(out=ov, in0=ov, in1=tv, op=OP.add)

            nc.sync.dma_start(
                out_d.ap().rearrange("(g p) f -> p g f", p=P),
                feats[:].rearrange("p (g f) -> p g f", f=FEAT),
            )

    nc.compile()
    return nc


_NC = None


def _get_nc():
    global _NC
    if _NC is None:
        _NC = build_bass()
    return _NC


def make_in_maps(fmap1, fmap2, centroids_coords):
    in_maps = []
    for core in range(8):
        bi, chunk = divmod(core, 4)
        m0 = chunk * NPIX
        f1 = np.ascontiguousarray(fmap1[bi].reshape(C, HW)[:, m0:m0 + NPIX], dtype=np.float32)
        f2 = np.ascontiguousarray(fmap2[bi].reshape(C, HW), dtype=np.float32)
        cc = centroids_coords[bi].reshape(2, HW)[:, m0:m0 + NPIX]
        ccx = np.ascontiguousarray(cc[0].reshape(NG, P).T, dtype=np.float32)  # [p, g]
        ccy = np.ascontiguousarray(cc[1].reshape(NG, P).T, dtype=np.float32)
        in_maps.append({"f1": f1, "f2": f2, "ccx": ccx, "ccy": ccy})
    return in_maps


def assemble(outs):
    """outs: list of 8 arrays [1024, 324] -> [2, 324, 64, 64]"""
    full = np.empty((2, FEAT, 64, 64), dtype=np.float32)
    for bi in range(2):
        feats = np.concatenate([outs[bi * 4 + c] for c in range(4)], axis=0)  # [4096, 324]
        full[bi] = feats.reshape(64, 64, FEAT).transpose(2, 0, 1)
    return full


def kernel(fmap1, fmap2, centroids_coords, trace=False):
    nc = _get_nc()
    in_maps = make_in_maps(fmap1, fmap2, centroids_coords)
    try:
        res = run_bass_kernel_spmd(nc, in_maps, core_ids=list(range(8)), trace=trace)
    except ModuleNotFoundError:
        res = run_bass_kernel_spmd(nc, in_maps, core_ids=list(range(8)), trace=False)
    out = assemble([r["out"] for r in res.results])
    if trace:
        kernel.last_result = res
    return out



# revision 7
# speedup vs baseline: 2.1090x; 2.1090x over previous
"""RAFT-style CorrBlock kernel for Trainium2 (8 NeuronCores, Bass/Tile).

Full inputs: fmap1 [2,256,64,64], fmap2 [2,256,64,64], centroids_coords [2,2,64,64].
Output: [2, 324, 64, 64] f32.

Sharding: data-parallel over the B*H1*W1 query-pixel axis. Core c handles batch
c//4, query pixels (c%4)*1024 .. +1024.

v2 (bf16 pipeline): matmuls in bf16 (1 cyc/row vs 4 for f32), corr pyramid slab
stored bf16 in per-group DRAM tensors (halves slab+gather traffic, and separate
tensors let group g's gather depend only on group g's single slab write so the
8 groups pipeline). Slab layout is per-pixel-interleaved: pixel p's 4 level
images live at p*5440 + {0,4096,5120,5376}, so each group needs ONE slab write.
Host pre-pools f2 (sums) and concatenates levels into one [256, 5440] operand.
Combine (mask + 4-tap bilinear) runs in bf16; output DMA'd bf16 and cast to f32
on host.
"""

import numpy as np
import ml_dtypes

import concourse.bass as bass
import concourse.bacc as bacc
import concourse.mybir as mybir
import concourse.tile as tile
from concourse.bass_utils import run_bass_kernel_spmd

f32 = mybir.dt.float32
bf16 = mybir.dt.bfloat16
i32 = mybir.dt.int32
OP = mybir.AluOpType

P = 128
C = 256
HW = 4096
NPIX = 1024
NG = NPIX // P     # 8 groups of 128 pixels
NLVL = 4
S = 9              # sample window side (2*RADIUS+1)
PS = 10            # patch side
W_L = [64, 32, 16, 8]
HW_L = [w * w for w in W_L]           # 4096, 1024, 256, 64
B_L = [9 * w + PS for w in W_L]       # band length: 586, 298, 154, 82
ROW = sum(HW_L)                       # 5440 = per-pixel slab row
RB = [0]
for _l in range(1, NLVL):
    RB.append(RB[-1] + HW_L[_l - 1])  # [0, 4096, 5120, 5376]
G1 = 512                              # guard elements at both slab ends
NT2 = G1 + P * ROW + G1
FEAT = NLVL * S * S                   # 324

# matmul chunking of the 5440 f2 columns: psum tiles of 1024 (2 banks) and the
# 320-wide tail (levels 2+3 fused)
CHUNKS = [(0, 1024), (1024, 2048), (2048, 3072), (3072, 4096),
          (4096, 5120), (5120, 5440)]


def _ap_view(t_ap, offset, dims):
    """Arbitrary strided view of a tile AP: dims = [[step, count], ...] free dims."""
    return bass.AP(t_ap.tensor, t_ap.offset + offset, [list(t_ap.ap[0])] + dims)


def build_bass():
    nc = bacc.Bacc("TRN2", target_bir_lowering=False, debug=False)

    f1_d = nc.dram_tensor("f1", [C, NPIX], bf16, kind="ExternalInput")
    f2_d = nc.dram_tensor("f2a", [C, ROW], bf16, kind="ExternalInput")
    ccx_d = nc.dram_tensor("ccx", [P, NG], f32, kind="ExternalInput")
    ccy_d = nc.dram_tensor("ccy", [P, NG], f32, kind="ExternalInput")
    out_d = nc.dram_tensor("out", [NPIX, FEAT], bf16, kind="ExternalOutput")
    slab_d = [nc.dram_tensor(f"slab{g}", [NT2], bf16) for g in range(NG)]

    with tile.TileContext(nc) as tc:
        with (
            tc.tile_pool(name="persist", bufs=1) as pp,
            tc.tile_pool(name="grp", bufs=2) as pg,
            tc.tile_pool(name="psA", bufs=3, space="PSUM") as psA,
            tc.tile_pool(name="psB", bufs=2, space="PSUM") as psB,
        ):
            # ---- guard zero-fill (avoid NaN poisoning of masked lanes) ----
            zt = pp.tile([1, G1], bf16, tag="zt")
            nc.vector.memset(zt[:], 0.0)
            for g in range(NG):
                eng = [nc.sync, nc.scalar][g % 2]
                eng.dma_start(slab_d[g].ap()[0:G1][None, :], zt[:])
                eng.dma_start(slab_d[g].ap()[NT2 - G1:NT2][None, :], zt[:])

            # ---- input loads ----
            f1t = []
            f2t = []
            for k in range(2):
                t1 = pp.tile([P, NPIX], bf16, tag=f"f1_{k}")
                nc.sync.dma_start(t1[:], f1_d.ap()[k * P:(k + 1) * P, :])
                f1t.append(t1)
                t2 = pp.tile([P, ROW], bf16, tag=f"f2_{k}")
                nc.sync.dma_start(t2[:], f2_d.ap()[k * P:(k + 1) * P, :])
                f2t.append(t2)
            ccx = pp.tile([P, NG], f32, tag="ccx")
            ccy = pp.tile([P, NG], f32, tag="ccy")
            nc.sync.dma_start(ccx[:], ccx_d.ap())
            nc.sync.dma_start(ccy[:], ccy_d.ap())

            # ---- per-level index / weight / mask precompute (all f32) ----
            kvi = pp.tile([P, NG * PS], i32, tag="kvi")
            nc.gpsimd.iota(kvi[:], pattern=[[0, NG], [1, PS]], base=0, channel_multiplier=0)
            kvf = pp.tile([P, NG * PS], f32, tag="kvf")
            nc.vector.tensor_copy(out=kvf[:], in_=kvi[:])
            # partition index p (same for every group), times ROW
            pf_i = pp.tile([P, NG], i32, tag="pf_i")
            nc.gpsimd.iota(pf_i[:], pattern=[[0, NG]], base=0, channel_multiplier=1)
            pfR = pp.tile([P, NG], f32, tag="pfR")
            nc.vector.tensor_copy(out=pfR[:], in_=pf_i[:])
            nc.vector.tensor_scalar_mul(pfR[:], pfR[:], float(ROW))

            idx_l = []
            w4 = [pp.tile([P, NG * NLVL], f32, tag=f"w4_{t}", name=f"w4_{t}")
                  for t in range(4)]
            m_l = []     # [P, NG*100] bf16 patch validity masks
            for l in range(NLVL):
                wl = W_L[l]
                inv = 1.0 / (1 << l)
                sc = 1.0 / (16.0 * (4.0 ** l))

                xs = pg.tile([P, NG], f32, tag="xs")
                ys = pg.tile([P, NG], f32, tag="ys")
                nc.vector.tensor_scalar_mul(xs[:], ccx[:], inv)
                nc.vector.tensor_scalar_mul(ys[:], ccy[:], inv)

                def floor_of(v, nm):
                    ti = pg.tile([P, NG], i32, tag=f"fl_i_{nm}")
                    nc.vector.tensor_copy(out=ti[:], in_=v[:])
                    tf = pg.tile([P, NG], f32, tag=f"fl_f_{nm}")
                    nc.vector.tensor_copy(out=tf[:], in_=ti[:])
                    gt = pg.tile([P, NG], f32, tag=f"fl_g_{nm}")
                    nc.vector.tensor_tensor(out=gt[:], in0=tf[:], in1=v[:], op=OP.is_gt)
                    fl = pg.tile([P, NG], f32, tag=f"fl_o_{nm}")
                    nc.vector.tensor_tensor(out=fl[:], in0=tf[:], in1=gt[:], op=OP.subtract)
                    return fl

                x0 = floor_of(xs, "x")
                y0 = floor_of(ys, "y")

                fx = pg.tile([P, NG], f32, tag="fx")
                fy = pg.tile([P, NG], f32, tag="fy")
                nc.vector.tensor_tensor(out=fx[:], in0=xs[:], in1=x0[:], op=OP.subtract)
                nc.vector.tensor_tensor(out=fy[:], in0=ys[:], in1=y0[:], op=OP.subtract)

                # weights: w_ab = wy_a * wx_b * sc ; wx1 = fx, wx0 = 1-fx
                wy0s = pg.tile([P, NG], f32, tag="wy0s")
                wy1s = pg.tile([P, NG], f32, tag="wy1s")
                nc.vector.tensor_scalar(wy0s[:], fy[:], -sc, sc, OP.mult, OP.add)
                nc.vector.tensor_scalar_mul(wy1s[:], fy[:], sc)
                wx0 = pg.tile([P, NG], f32, tag="wx0")
                nc.vector.tensor_scalar(wx0[:], fx[:], -1.0, 1.0, OP.mult, OP.add)
                for t, (wya, wxb) in enumerate(
                    ((wy0s, wx0), (wy0s, fx), (wy1s, wx0), (wy1s, fx))
                ):
                    # layout [P, (g l)]: stride NLVL per group, offset l
                    nc.vector.tensor_tensor(
                        out=_ap_view(w4[t][:], l, [[NLVL, NG]]),
                        in0=wya[:], in1=wxb[:], op=OP.mult,
                    )

                # band start index: p*ROW + RB_l + (y0-4)*W_l + (x0-4)
                t1 = pg.tile([P, NG], f32, tag="idx_t1")
                nc.vector.scalar_tensor_tensor(
                    out=t1[:], in0=y0[:], scalar=float(wl), in1=x0[:],
                    op0=OP.mult, op1=OP.add,
                )
                t2 = pg.tile([P, NG], f32, tag="idx_t2")
                nc.vector.tensor_tensor(out=t2[:], in0=pfR[:], in1=t1[:], op=OP.add)
                # fold the +G1 guard offset in here: HW DGE drops descriptors with
                # negative raw indices (sim applies element_offset first, HW not)
                t3 = pg.tile([P, NG], f32, tag="idx_t3")
                nc.vector.tensor_scalar_add(t3[:], t2[:], float(G1 + RB[l] - 4 * wl - 4))
                ii = pp.tile([P, NG], i32, tag=f"idx_{l}")
                nc.vector.tensor_copy(out=ii[:], in_=t3[:])
                idx_l.append(ii)

                # row/col validity: valid iff 4-k <= c0 <= wl+3-k
                def valid(c0, lim, nm):
                    tt = pg.tile([P, NG * PS], f32, tag=f"v_t_{nm}")
                    nc.vector.tensor_tensor(
                        out=tt[:].rearrange("p (g k) -> p g k", k=PS),
                        in0=kvf[:].rearrange("p (g k) -> p g k", k=PS),
                        in1=c0[:, :, None].to_broadcast([P, NG, PS]),
                        op=OP.add,
                    )
                    c1 = pg.tile([P, NG * PS], f32, tag=f"v_c_{nm}")
                    nc.vector.tensor_scalar(c1[:], tt[:], 4.0, None, OP.is_ge)
                    vv = pg.tile([P, NG * PS], f32, tag=f"v_o_{nm}")
                    nc.vector.scalar_tensor_tensor(
                        out=vv[:], in0=tt[:], scalar=float(lim + 3), in1=c1[:],
                        op0=OP.is_le, op1=OP.mult,
                    )
                    return vv

                rv = valid(y0, wl, "r")
                cv = valid(x0, wl, "c")
                mm = pp.tile([P, NG * PS * PS], bf16, tag=f"m_{l}")
                nc.vector.tensor_tensor(
                    out=mm[:].rearrange("p (g a b) -> p g a b", a=PS, b=PS),
                    in0=rv[:].rearrange("p (g k) -> p g k", k=PS)[:, :, :, None]
                        .to_broadcast([P, NG, PS, PS]),
                    in1=cv[:].rearrange("p (g k) -> p g k", k=PS)[:, :, None, :]
                        .to_broadcast([P, NG, PS, PS]),
                    op=OP.mult,
                )
                m_l.append(mm)

            # ---- main pipeline over the 8 groups ----
            for g in range(NG):
                corr = pg.tile([P, ROW], bf16, tag="corr", name=f"corr_{g}")
                for ci, (c0, c1) in enumerate(CHUNKS):
                    wid = c1 - c0
                    pool = psA if wid == 1024 else psB
                    pt = pool.tile([P, wid], f32, tag="mmA" if wid == 1024 else "mmB")
                    for s0 in range(0, wid, 512):
                        sw = min(512, wid - s0)
                        for k in range(2):
                            nc.tensor.matmul(
                                out=pt[:, s0:s0 + sw],
                                lhsT=f1t[k][:, g * P:(g + 1) * P],
                                rhs=f2t[k][:, c0 + s0:c0 + s0 + sw],
                                start=(k == 0),
                                stop=(k == 1),
                            )
                    # PSUM -> SBUF downcast copy; L0 chunks on scalar, rest on vector
                    if ci < 4:
                        nc.scalar.copy(out=corr[:, c0:c1], in_=pt[:])
                    else:
                        nc.vector.tensor_copy(out=corr[:, c0:c1], in_=pt[:])

                # one slab write for the whole group
                nc.sync.dma_start(
                    slab_d[g].ap()[G1:G1 + P * ROW].rearrange("(p f) -> p f", f=ROW),
                    corr[:],
                )

                # band gathers (one per level) + masked patches
                pm4 = pg.tile([P, NLVL * PS * PS], bf16, tag="pm4", name=f"pm4_{g}")
                for l in range(NLVL):
                    bl = B_L[l]
                    wl = W_L[l]
                    band = pg.tile([P, bl], bf16, tag=f"band_{l}", name=f"band_{l}_{g}")
                    nc.gpsimd.indirect_dma_start(
                        out=band[:],
                        out_offset=None,
                        in_=slab_d[g].ap()[:, None],
                        in_offset=bass.IndirectOffsetOnAxis(ap=idx_l[l][:, g:g + 1], axis=0),
                        element_offset=0,
                    )
                    nc.vector.tensor_tensor(
                        out=_ap_view(pm4[:], l * PS * PS, [[PS, PS], [1, PS]]),
                        in0=_ap_view(band[:], 0, [[wl, PS], [1, PS]]),
                        in1=_ap_view(m_l[l][:], g * PS * PS, [[PS, PS], [1, PS]]),
                        op=OP.mult,
                    )

                # 4-tap bilinear combine, all bf16
                feats = pg.tile([P, FEAT], bf16, tag="feats", name=f"feats_{g}")
                ov = _ap_view(feats[:], 0, [[S * S, NLVL], [S, S], [1, S]])
                for t, (a, b) in enumerate(((0, 0), (0, 1), (1, 0), (1, 1))):
                    # feature index = i*9 + j with i = x-offset (outer), j = y-offset
                    # (inner); patch element [y=j+a, x=i+b] sits at (j+a)*10 + (i+b)
                    pv = _ap_view(pm4[:], a * PS + b, [[PS * PS, NLVL], [1, S], [PS, S]])
                    wb = _ap_view(w4[t][:], g * NLVL,
                                  [[1, NLVL], [0, S], [0, S]])
                    if t == 0:
                        nc.vector.tensor_tensor(out=ov, in0=pv, in1=wb, op=OP.mult)
                    else:
                        tmp = pg.tile([P, FEAT], bf16, tag="cmb_tmp")
                        tv = _ap_view(tmp[:], 0, [[S * S, NLVL], [S, S], [1, S]])
                        nc.vector.tensor_tensor(out=tv, in0=pv, in1=wb, op=OP.mult)
                        nc.vector.tensor_tensor(out=ov, in0=ov, in1=tv, op=OP.add)

                nc.scalar.dma_start(out_d.ap()[g * P:(g + 1) * P, :], feats[:])

    nc.compile()
    return nc


_NC = None


def _get_nc():
    global _NC
    if _NC is None:
        _NC = build_bass()
    return _NC


def _pool_f2(f2b):
    """f2b: [C, 64, 64] f32 -> [C, 5440] level-concatenated pooled SUMS."""
    lvls = [f2b.reshape(C, HW)]
    cur = f2b
    for _ in range(1, NLVL):
        c, h, w = cur.shape
        cur = cur.reshape(c, h // 2, 2, w // 2, 2).sum(axis=(2, 4))
        lvls.append(cur.reshape(C, -1))
    return np.concatenate(lvls, axis=1)


def make_in_maps(fmap1, fmap2, centroids_coords):
    bf = ml_dtypes.bfloat16
    f2a = [np.ascontiguousarray(_pool_f2(np.asarray(fmap2[bi], dtype=np.float32))).astype(bf)
           for bi in range(2)]
    in_maps = []
    for core in range(8):
        bi, chunk = divmod(core, 4)
        m0 = chunk * NPIX
        f1 = np.ascontiguousarray(
            fmap1[bi].reshape(C, HW)[:, m0:m0 + NPIX]).astype(bf)
        cc = centroids_coords[bi].reshape(2, HW)[:, m0:m0 + NPIX]
        ccx = np.ascontiguousarray(cc[0].reshape(NG, P).T, dtype=np.float32)  # [p, g]
        ccy = np.ascontiguousarray(cc[1].reshape(NG, P).T, dtype=np.float32)
        in_maps.append({"f1": f1, "f2a": f2a[bi], "ccx": ccx, "ccy": ccy})
    return in_maps


def assemble(outs):
    """outs: list of 8 arrays [1024, 324] bf16 -> [2, 324, 64, 64] f32"""
    full = np.empty((2, FEAT, 64, 64), dtype=np.float32)
    for bi in range(2):
        feats = np.concatenate(
            [np.asarray(outs[bi * 4 + c], dtype=np.float32) for c in range(4)], axis=0)
        full[bi] = feats.reshape(64, 64, FEAT).transpose(2, 0, 1)
    return full


def kernel(fmap1, fmap2, centroids_coords, trace=False):
    nc = _get_nc()
    in_maps = make_in_maps(fmap1, fmap2, centroids_coords)
    try:
        res = run_bass_kernel_spmd(nc, in_maps, core_ids=list(range(8)), trace=trace)
    except ModuleNotFoundError:
        res = run_bass_kernel_spmd(nc, in_maps, core_ids=list(range(8)), trace=False)
    out = assemble([r["out"] for r in res.results])
    if trace:
        kernel.last_result = res
    return out


# revision 8
# speedup vs baseline: 2.6674x; 1.2648x over previous
"""RAFT-style CorrBlock kernel for Trainium2 (8 NeuronCores, Bass/Tile).

Full inputs: fmap1 [2,256,64,64], fmap2 [2,256,64,64], centroids_coords [2,2,64,64].
Output: [2, 324, 64, 64] f32.

Sharding: data-parallel over the B*H1*W1 query-pixel axis. Core c handles batch
c//4, query pixels (c%4)*1024 .. +1024.

v3: bf16 matmul/slab/combine pipeline, two DRAM slab tensors per pixel-group
(levels 1-3 written first so their band gathers stream while the level-0 chunks
are still on the PE), inputs loaded small-first so the first matmul starts ~4us
in, indirect-gather indices pre-offset by the guard size (HW DGE drops negative
raw indices). Host pre-pools f2 (sums) into one [256, 5440] bf16 operand and
casts the bf16 output back to f32.
"""

import numpy as np
import ml_dtypes

import concourse.bass as bass
import concourse.bacc as bacc
import concourse.mybir as mybir
import concourse.tile as tile
from concourse.bass_utils import run_bass_kernel_spmd

f32 = mybir.dt.float32
bf16 = mybir.dt.bfloat16
i32 = mybir.dt.int32
OP = mybir.AluOpType

P = 128
C = 256
HW = 4096
NPIX = 1024
NG = NPIX // P     # 8 groups of 128 pixels
NLVL = 4
S = 9              # sample window side (2*RADIUS+1)
PS = 10            # patch side
W_L = [64, 32, 16, 8]
HW_L = [w * w for w in W_L]           # 4096, 1024, 256, 64
B_L = [9 * w + PS for w in W_L]       # band length: 586, 298, 154, 82
FEAT = NLVL * S * S                   # 324
G1 = 512                              # guard elements at both slab ends

ROWA = HW_L[0]                        # 4096 (level-0 slab row per pixel)
ROWB = HW_L[1] + HW_L[2] + HW_L[3]    # 1344 (levels 1-3 slab row per pixel)
RBB = [0, HW_L[1], HW_L[1] + HW_L[2]]  # level offsets within the B row
NTA = G1 + P * ROWA + G1
NTB = G1 + P * ROWB + G1
F2COLS = sum(HW_L)                    # 5440 in the concatenated f2 operand


def _ap_view(t_ap, offset, dims):
    """Arbitrary strided view of a tile AP: dims = [[step, count], ...] free dims."""
    return bass.AP(t_ap.tensor, t_ap.offset + offset, [list(t_ap.ap[0])] + dims)


def build_bass():
    nc = bacc.Bacc("TRN2", target_bir_lowering=False, debug=False)

    f1_d = nc.dram_tensor("f1", [C, NPIX], bf16, kind="ExternalInput")
    f2_d = nc.dram_tensor("f2a", [C, F2COLS], bf16, kind="ExternalInput")
    ccx_d = nc.dram_tensor("ccx", [P, NG], f32, kind="ExternalInput")
    ccy_d = nc.dram_tensor("ccy", [P, NG], f32, kind="ExternalInput")
    out_d = nc.dram_tensor("out", [NPIX, FEAT], bf16, kind="ExternalOutput")
    slabA = [nc.dram_tensor(f"slabA{g}", [NTA], bf16) for g in range(NG)]
    slabB = [nc.dram_tensor(f"slabB{g}", [NTB], bf16) for g in range(NG)]

    with tile.TileContext(nc) as tc:
        with (
            tc.tile_pool(name="persist", bufs=1) as pp,
            tc.tile_pool(name="grp", bufs=2) as pg,
            tc.tile_pool(name="psA", bufs=3, space="PSUM") as psA,
            tc.tile_pool(name="psB", bufs=2, space="PSUM") as psB,
        ):
            # ---- input loads, small and L123-f2 first so compute starts early ----
            ccx = pp.tile([P, NG], f32, tag="ccx")
            ccy = pp.tile([P, NG], f32, tag="ccy")
            nc.sync.dma_start(ccx[:], ccx_d.ap())
            nc.sync.dma_start(ccy[:], ccy_d.ap())
            f1t = []
            for k in range(2):
                t1 = pp.tile([P, NPIX], bf16, tag=f"f1_{k}", name=f"f1_{k}")
                nc.sync.dma_start(t1[:], f1_d.ap()[k * P:(k + 1) * P, :])
                f1t.append(t1)
            f2B = []
            for k in range(2):
                tb = pp.tile([P, ROWB], bf16, tag=f"f2B_{k}", name=f"f2B_{k}")
                nc.sync.dma_start(tb[:], f2_d.ap()[k * P:(k + 1) * P, ROWA:F2COLS])
                f2B.append(tb)
            f2A = []
            for k in range(2):
                ta = pp.tile([P, ROWA], bf16, tag=f"f2A_{k}", name=f"f2A_{k}")
                nc.sync.dma_start(ta[:], f2_d.ap()[k * P:(k + 1) * P, 0:ROWA])
                f2A.append(ta)

            # ---- guard zero-fill: one strided [2, G1] DMA per slab tensor ----
            zt = pp.tile([1, 2 * G1], bf16, tag="zt")
            nc.vector.memset(zt[:], 0.0)
            for g in range(NG):
                nc.sync.dma_start(
                    _ap_view(slabA[g].ap()[:, None], 0, [[NTA - G1, 2], [1, G1]])[0],
                    zt[:].rearrange("o (t x) -> o t x", t=2)[0],
                )
                nc.sync.dma_start(
                    _ap_view(slabB[g].ap()[:, None], 0, [[NTB - G1, 2], [1, G1]])[0],
                    zt[:].rearrange("o (t x) -> o t x", t=2)[0],
                )

            # ---- per-level index / weight / mask precompute (all f32) ----
            kvi = pp.tile([P, NG * PS], i32, tag="kvi")
            nc.gpsimd.iota(kvi[:], pattern=[[0, NG], [1, PS]], base=0, channel_multiplier=0)
            kvf = pp.tile([P, NG * PS], f32, tag="kvf")
            nc.vector.tensor_copy(out=kvf[:], in_=kvi[:])
            pf_i = pp.tile([P, NG], i32, tag="pf_i")
            nc.gpsimd.iota(pf_i[:], pattern=[[0, NG]], base=0, channel_multiplier=1)
            pf_f = pp.tile([P, NG], f32, tag="pf_f")
            nc.vector.tensor_copy(out=pf_f[:], in_=pf_i[:])

            idx_l = []
            w4 = [pp.tile([P, NG * NLVL], f32, tag=f"w4_{t}", name=f"w4_{t}")
                  for t in range(4)]
            m_l = []     # [P, NG*100] bf16 patch validity masks
            for l in range(NLVL):
                wl = W_L[l]
                inv = 1.0 / (1 << l)
                sc = 1.0 / (16.0 * (4.0 ** l))
                rowl = ROWA if l == 0 else ROWB
                basel = G1 + (0 if l == 0 else RBB[l - 1]) - 4 * wl - 4

                xs = pg.tile([P, NG], f32, tag="xs")
                ys = pg.tile([P, NG], f32, tag="ys")
                nc.vector.tensor_scalar_mul(xs[:], ccx[:], inv)
                nc.vector.tensor_scalar_mul(ys[:], ccy[:], inv)

                def floor_of(v, nm):
                    ti = pg.tile([P, NG], i32, tag=f"fl_i_{nm}")
                    nc.vector.tensor_copy(out=ti[:], in_=v[:])
                    tf = pg.tile([P, NG], f32, tag=f"fl_f_{nm}")
                    nc.vector.tensor_copy(out=tf[:], in_=ti[:])
                    gt = pg.tile([P, NG], f32, tag=f"fl_g_{nm}")
                    nc.vector.tensor_tensor(out=gt[:], in0=tf[:], in1=v[:], op=OP.is_gt)
                    fl = pg.tile([P, NG], f32, tag=f"fl_o_{nm}")
                    nc.vector.tensor_tensor(out=fl[:], in0=tf[:], in1=gt[:], op=OP.subtract)
                    return fl

                x0 = floor_of(xs, "x")
                y0 = floor_of(ys, "y")

                fx = pg.tile([P, NG], f32, tag="fx")
                fy = pg.tile([P, NG], f32, tag="fy")
                nc.vector.tensor_tensor(out=fx[:], in0=xs[:], in1=x0[:], op=OP.subtract)
                nc.vector.tensor_tensor(out=fy[:], in0=ys[:], in1=y0[:], op=OP.subtract)

                # weights: w_ab = wy_a * wx_b * sc ; wx1 = fx, wx0 = 1-fx
                wy0s = pg.tile([P, NG], f32, tag="wy0s")
                wy1s = pg.tile([P, NG], f32, tag="wy1s")
                nc.vector.tensor_scalar(wy0s[:], fy[:], -sc, sc, OP.mult, OP.add)
                nc.vector.tensor_scalar_mul(wy1s[:], fy[:], sc)
                wx0 = pg.tile([P, NG], f32, tag="wx0")
                nc.vector.tensor_scalar(wx0[:], fx[:], -1.0, 1.0, OP.mult, OP.add)
                for t, (wya, wxb) in enumerate(
                    ((wy0s, wx0), (wy0s, fx), (wy1s, wx0), (wy1s, fx))
                ):
                    # layout [P, (g l)]: stride NLVL per group, offset l
                    nc.vector.tensor_tensor(
                        out=_ap_view(w4[t][:], l, [[NLVL, NG]]),
                        in0=wya[:], in1=wxb[:], op=OP.mult,
                    )

                # band start: G1 + p*row + RB + (y0-4)*W_l + (x0-4)  (always >= 0:
                # HW DGE silently drops descriptors with negative raw indices)
                t1 = pg.tile([P, NG], f32, tag="idx_t1")
                nc.vector.scalar_tensor_tensor(
                    out=t1[:], in0=y0[:], scalar=float(wl), in1=x0[:],
                    op0=OP.mult, op1=OP.add,
                )
                t2 = pg.tile([P, NG], f32, tag="idx_t2")
                nc.vector.scalar_tensor_tensor(
                    out=t2[:], in0=pf_f[:], scalar=float(rowl), in1=t1[:],
                    op0=OP.mult, op1=OP.add,
                )
                t3 = pg.tile([P, NG], f32, tag="idx_t3")
                nc.vector.tensor_scalar_add(t3[:], t2[:], float(basel))
                ii = pp.tile([P, NG], i32, tag=f"idx_{l}")
                nc.vector.tensor_copy(out=ii[:], in_=t3[:])
                idx_l.append(ii)

                # row/col validity: valid iff 4-k <= c0 <= wl+3-k
                def valid(c0, lim, nm):
                    tt = pg.tile([P, NG * PS], f32, tag=f"v_t_{nm}")
                    nc.vector.tensor_tensor(
                        out=tt[:].rearrange("p (g k) -> p g k", k=PS),
                        in0=kvf[:].rearrange("p (g k) -> p g k", k=PS),
                        in1=c0[:, :, None].to_broadcast([P, NG, PS]),
                        op=OP.add,
                    )
                    c1 = pg.tile([P, NG * PS], f32, tag=f"v_c_{nm}")
                    nc.vector.tensor_scalar(c1[:], tt[:], 4.0, None, OP.is_ge)
                    vv = pg.tile([P, NG * PS], f32, tag=f"v_o_{nm}")
                    nc.vector.scalar_tensor_tensor(
                        out=vv[:], in0=tt[:], scalar=float(lim + 3), in1=c1[:],
                        op0=OP.is_le, op1=OP.mult,
                    )
                    return vv

                rv = valid(y0, wl, "r")
                cv = valid(x0, wl, "c")
                mm = pp.tile([P, NG * PS * PS], bf16, tag=f"m_{l}")
                nc.vector.tensor_tensor(
                    out=mm[:].rearrange("p (g a b) -> p g a b", a=PS, b=PS),
                    in0=rv[:].rearrange("p (g k) -> p g k", k=PS)[:, :, :, None]
                        .to_broadcast([P, NG, PS, PS]),
                    in1=cv[:].rearrange("p (g k) -> p g k", k=PS)[:, :, None, :]
                        .to_broadcast([P, NG, PS, PS]),
                    op=OP.mult,
                )
                m_l.append(mm)

            # ---- main pipeline over the 8 groups ----
            for g in range(NG):
                # levels 1-3 first: small matmuls, slabB write, gathers stream
                # while the level-0 chunks run on the PE
                corrB = pg.tile([P, ROWB], bf16, tag="corrB", name=f"corrB_{g}")
                ptB1 = psA.tile([P, 1024], f32, tag="mmA", name=f"mmB1_{g}")
                for s0 in (0, 512):
                    for k in range(2):
                        nc.tensor.matmul(
                            out=ptB1[:, s0:s0 + 512],
                            lhsT=f1t[k][:, g * P:(g + 1) * P],
                            rhs=f2B[k][:, s0:s0 + 512],
                            start=(k == 0), stop=(k == 1),
                        )
                nc.scalar.copy(out=corrB[:, 0:1024], in_=ptB1[:])
                ptB2 = psB.tile([P, 320], f32, tag="mmB", name=f"mmB2_{g}")
                for k in range(2):
                    nc.tensor.matmul(
                        out=ptB2[:],
                        lhsT=f1t[k][:, g * P:(g + 1) * P],
                        rhs=f2B[k][:, 1024:1344],
                        start=(k == 0), stop=(k == 1),
                    )
                nc.vector.tensor_copy(out=corrB[:, 1024:1344], in_=ptB2[:])
                nc.sync.dma_start(
                    slabB[g].ap()[G1:G1 + P * ROWB].rearrange("(p f) -> p f", f=ROWB),
                    corrB[:],
                )

                # level 0: four 1024-wide chunks
                corrA = pg.tile([P, ROWA], bf16, tag="corrA", name=f"corrA_{g}")
                for ci in range(4):
                    c0 = ci * 1024
                    pt = psA.tile([P, 1024], f32, tag="mmA", name=f"mmA_{g}_{ci}")
                    for s0 in (0, 512):
                        for k in range(2):
                            nc.tensor.matmul(
                                out=pt[:, s0:s0 + 512],
                                lhsT=f1t[k][:, g * P:(g + 1) * P],
                                rhs=f2A[k][:, c0 + s0:c0 + s0 + 512],
                                start=(k == 0), stop=(k == 1),
                            )
                    if ci < 3:
                        nc.scalar.copy(out=corrA[:, c0:c0 + 1024], in_=pt[:])
                    else:
                        nc.vector.tensor_copy(out=corrA[:, c0:c0 + 1024], in_=pt[:])
                nc.sync.dma_start(
                    slabA[g].ap()[G1:G1 + P * ROWA].rearrange("(p f) -> p f", f=ROWA),
                    corrA[:],
                )

                # band gathers (levels 1-3 from slabB first, then level 0) + pm
                pm4 = pg.tile([P, NLVL * PS * PS], bf16, tag="pm4", name=f"pm4_{g}")
                for l in (1, 2, 3, 0):
                    bl = B_L[l]
                    wl = W_L[l]
                    src = slabA[g] if l == 0 else slabB[g]
                    band = pg.tile([P, bl], bf16, tag=f"band_{l}", name=f"band_{l}_{g}")
                    nc.gpsimd.indirect_dma_start(
                        out=band[:],
                        out_offset=None,
                        in_=src.ap()[:, None],
                        in_offset=bass.IndirectOffsetOnAxis(ap=idx_l[l][:, g:g + 1], axis=0),
                        element_offset=0,
                    )
                    nc.vector.tensor_tensor(
                        out=_ap_view(pm4[:], l * PS * PS, [[PS, PS], [1, PS]]),
                        in0=_ap_view(band[:], 0, [[wl, PS], [1, PS]]),
                        in1=_ap_view(m_l[l][:], g * PS * PS, [[PS, PS], [1, PS]]),
                        op=OP.mult,
                    )

                # 4-tap bilinear combine, all bf16
                feats = pg.tile([P, FEAT], bf16, tag="feats", name=f"feats_{g}")
                ov = _ap_view(feats[:], 0, [[S * S, NLVL], [S, S], [1, S]])
                for t, (a, b) in enumerate(((0, 0), (0, 1), (1, 0), (1, 1))):
                    # feature index = i*9 + j with i = x-offset (outer), j = y-offset
                    # (inner); patch element [y=j+a, x=i+b] sits at (j+a)*10 + (i+b)
                    pv = _ap_view(pm4[:], a * PS + b, [[PS * PS, NLVL], [1, S], [PS, S]])
                    wb = _ap_view(w4[t][:], g * NLVL,
                                  [[1, NLVL], [0, S], [0, S]])
                    if t == 0:
                        nc.vector.tensor_tensor(out=ov, in0=pv, in1=wb, op=OP.mult)
                    else:
                        tmp = pg.tile([P, FEAT], bf16, tag="cmb_tmp")
                        tv = _ap_view(tmp[:], 0, [[S * S, NLVL], [S, S], [1, S]])
                        nc.vector.tensor_tensor(out=tv, in0=pv, in1=wb, op=OP.mult)
                        nc.vector.tensor_tensor(out=ov, in0=ov, in1=tv, op=OP.add)

                nc.scalar.dma_start(out_d.ap()[g * P:(g + 1) * P, :], feats[:])

    nc.compile()
    return nc


_NC = None


def _get_nc():
    global _NC
    if _NC is None:
        _NC = build_bass()
    return _NC


def _pool_f2(f2b):
    """f2b: [C, 64, 64] f32 -> [C, 5440] level-concatenated pooled SUMS."""
    lvls = [f2b.reshape(C, HW)]
    cur = f2b
    for _ in range(1, NLVL):
        c, h, w = cur.shape
        cur = cur.reshape(c, h // 2, 2, w // 2, 2).sum(axis=(2, 4))
        lvls.append(cur.reshape(c, -1))
    return np.concatenate(lvls, axis=1)


def make_in_maps(fmap1, fmap2, centroids_coords):
    bf = ml_dtypes.bfloat16
    f2a = [np.ascontiguousarray(_pool_f2(np.asarray(fmap2[bi], dtype=np.float32))).astype(bf)
           for bi in range(2)]
    in_maps = []
    for core in range(8):
        bi, chunk = divmod(core, 4)
        m0 = chunk * NPIX
        f1 = np.ascontiguousarray(
            fmap1[bi].reshape(C, HW)[:, m0:m0 + NPIX]).astype(bf)
        cc = centroids_coords[bi].reshape(2, HW)[:, m0:m0 + NPIX]
        ccx = np.ascontiguousarray(cc[0].reshape(NG, P).T, dtype=np.float32)  # [p, g]
        ccy = np.ascontiguousarray(cc[1].reshape(NG, P).T, dtype=np.float32)
        in_maps.append({"f1": f1, "f2a": f2a[bi], "ccx": ccx, "ccy": ccy})
    return in_maps


def assemble(outs):
    """outs: list of 8 arrays [1024, 324] bf16 -> [2, 324, 64, 64] f32"""
    full = np.empty((2, FEAT, 64, 64), dtype=np.float32)
    for bi in range(2):
        feats = np.concatenate(
            [np.asarray(outs[bi * 4 + c], dtype=np.float32) for c in range(4)], axis=0)
        full[bi] = feats.reshape(64, 64, FEAT).transpose(2, 0, 1)
    return full


def kernel(fmap1, fmap2, centroids_coords, trace=False):
    nc = _get_nc()
    in_maps = make_in_maps(fmap1, fmap2, centroids_coords)
    try:
        res = run_bass_kernel_spmd(nc, in_maps, core_ids=list(range(8)), trace=trace)
    except ModuleNotFoundError:
        res = run_bass_kernel_spmd(nc, in_maps, core_ids=list(range(8)), trace=False)
    out = assemble([r["out"] for r in res.results])
    if trace:
        kernel.last_result = res
    return out
